# revision 27
# baseline (speedup 1.0000x reference)
"""TextCNN-style conv layer (kernel sizes 3/4/5, EMB=300 -> DEPTH=256, bias,
ReLU, max-pool over time) as a Bass/Tile kernel for 8 Trainium2 NeuronCores.

Strategy: data-parallel over batch (8 samples per core), weights replicated.

Conv as dense-K matmuls over the im2col matrix Xrep[k, i] = x[i + k//300,
k%300] in fp8 e4m3 with perf_mode=DoubleRow: each matmul contracts 256
virtual K-rows (two 128-row blocks presented as [128, 2, N] APs, 2 fp8 MACs
per PE cell per cycle), so a branch needs ceil(n*300/256) matmuls (4/5/6 for
n=3/4/5).  Measured MM pace is N/2.4GHz (165ns at N=390) with LDWEIGHTS
fully hidden, i.e. a 240-matmul / ~40us PE floor per core.  Branch
boundaries inside a super-tile are zero-padded in the *weights*.

End-to-end e4m3 quantization error on this distribution measures ~1.2e-2
relative L2, inside the 2e-2 gate.

Schedule (from trace analysis of v1):
- Phases (branch, sample-half); all s0-3 phases first so the early DMA only
  races compute for half the x bytes.  Within a phase, per-(sample,
  depth-half) K-contiguous matmul chains -> stop-matmuls arrive ~1us apart
  and the DVE reduce_max (561ns each) never bursts or stalls the PE.
- x is packed as 3 tiles per sample ([128, 4, SEQP] fp8 = two DoubleRow
  super-tiles each) to cut DMA/semaphore/teardown count while keeping the
  first-need granularity small; issue order == first-need order on the two
  fast HWDGE rings; br1/br2 weights + bias ride the gpsimd SWDGE ring.
- ~10 junk matmuls on a memset tile warm the PE HAM clock gate (1.2->2.4
  GHz) during the unavoidable initial DMA wait.

Epilogue: relu(max_i(y + b)) == max(0, max_i y + b): DVE reduce_max over the
window axis straight out of PSUM, broadcast bias add + clamp at 0, output
staged [d, branch, half, sample] per core and de-transposed on host.
"""

import numpy as np
import ml_dtypes

B, SEQ, EMB = 64, 394, 300
DEPTH = 256
NCORES = 8
BPC = B // NCORES  # samples per core
SEQP = 400  # free-dim padded (zeros) so shifted loads stay in bounds
NS = (3, 4, 5)
NT2 = (4, 5, 6)  # ceil(n*300/256) 256-row K-super-tiles per branch
CTB = (0, 4, 9)  # weight super-tile base per branch in the packed array
NCT = 15
KCT = 6  # distinct Xrep 256-row super-tiles per sample
XG = 3  # x tile groups per sample (2 super-tiles each)
NWARM = 8

# On-chip im2col: x ships to HBM unreplicated as [128, 3, SEQP] (row e of the
# transposed sample at partition e%128, block e//128; rows >= 300 zero), and
# the 12 replicated 128-row K-tiles are built in SBUF by shifted SBUF->SBUF
# segment copies.  Segment (r, pd0, b, ps0, j, run): K-tile r partitions
# [pd0, pd0+run) <- base block b partitions [ps0, ps0+run) shifted j columns.
# The final segment over-reads into the base tile's zero padding so K-tile 11
# rows 1500-1535 come out zero (no memset needed).
_XSEGS = []
for _r in range(12):
    _k, _k1 = 128 * _r, 128 * (_r + 1)
    while _k < _k1:
        _j, _e = divmod(_k, EMB)
        _run = min(_k1 - _k, EMB - _e, 128 - (_e % 128))
        if _r == 11 and _k + _run == 1500:
            _run = _k1 - _k  # extend into zero padding rows
        _XSEGS.append((_r, _k - 128 * _r, _e // 128, _e % 128, _j, _run))
        _k += _run

TRACE = False
LAST_RESULT = None

_built = None


def _build_bass():
    import concourse.mybir as mybir
    import concourse.tile as tile
    from concourse import bacc
    from contextlib import ExitStack

    f32 = mybir.dt.float32
    f8 = mybir.dt.float8e4
    DR = mybir.MatmulPerfMode.DoubleRow

    nc = bacc.Bacc("TRN2", target_bir_lowering=False)
    xb_d = nc.dram_tensor("xb", (BPC, 128, 3, SEQP), f8, kind="ExternalInput")
    wq_d = nc.dram_tensor("wq", (128, 2, NCT, 2, 128), f8, kind="ExternalInput")
    bp_d = nc.dram_tensor("bp", (128, 3, 2), f32, kind="ExternalInput")
    out_d = nc.dram_tensor("out_t", (128, 3, 2, BPC), f32, kind="ExternalOutput")

    with tile.TileContext(nc) as tc, ExitStack() as ctx:
        xpool = ctx.enter_context(tc.tile_pool(name="x", bufs=1))
        xbpool = ctx.enter_context(tc.tile_pool(name="xb", bufs=1))
        wpool = ctx.enter_context(tc.tile_pool(name="w", bufs=1))
        cpool = ctx.enter_context(tc.tile_pool(name="consts", bufs=1))
        spool = ctx.enter_context(tc.tile_pool(name="stage", bufs=1))
        pspool = ctx.enter_context(tc.tile_pool(name="ps", bufs=8, space="PSUM"))

        # PE warmup: junk DoubleRow matmuls on a zeroed tile, no DMA deps.
        # The PSUM bank is never read; the first real matmul on that bank
        # uses start=True, which overwrites.
        wz = cpool.tile([128, 2, SEQP], f8)
        nc.gpsimd.memset(wz[:], 0)
        psw = pspool.tile([128, 512], f32, tag="ps", name="ps_warm")
        for i in range(NWARM):
            nc.tensor.matmul(
                psw[:, :390],
                lhsT=wz[:, :, :128],
                rhs=wz[:, :, :390],
                start=True,
                stop=True,
                perf_mode=DR,
            )

        # Single DMA ring: the Tile runtime allocates semaphores per
        # (ring x DMA queue); using one ring cuts the fixed start/end
        # semaphore-clear chains (~57 -> ~25 waits per engine).
        ring = nc.sync

        wts = {}

        def load_w(dh, br):
            nt = NT2[br]
            wt = wpool.tile([128, nt, 2, 128], f8, tag=f"w{dh}{br}")
            ring.dma_start(wt[:], wq_d[:, dh, CTB[br] : CTB[br] + nt])
            wts[dh, br] = wt

        def lhsT_for(dh, br, ct):
            return wts[dh, br][:, ct]

        # Unreplicated x loads on the HBM ring; im2col expansion as shifted
        # SBUF->SBUF segment copies on the scalar+gpsimd rings.
        xbase = {}

        def load_xb(s):
            t = xbpool.tile([128, 3, SEQP], f8, tag=f"xb{s}")
            ring.dma_start(t[:], xb_d[s])
            xbase[s] = t

        xts = {}
        xrr = [0]
        xrings = (nc.scalar, nc.gpsimd)

        def expand_x(s, g):
            t = xpool.tile([128, 4, SEQP], f8, tag=f"x{s}_{g}")
            for r, pd0, b, ps0, j, run in _XSEGS:
                if r // 4 != g:
                    continue
                eng = xrings[xrr[0] % 2]
                xrr[0] += 1
                eng.dma_start(
                    t[pd0 : pd0 + run, r % 4, 0 : SEQP - j],
                    xbase[s][ps0 : ps0 + run, b, j:SEQP],
                )
            xts[s, g] = t

        # DMA issue order == first-need order.
        load_w(0, 0)
        load_xb(0)
        load_w(1, 0)
        for s in range(1, 4):
            load_xb(s)
        load_w(0, 1)
        load_w(1, 1)
        load_w(0, 2)
        load_w(1, 2)
        for s in range(4, 8):
            load_xb(s)
        bt = cpool.tile([128, 3, 2], f32)
        ring.dma_start(bt[:], bp_d[:])

        for s in range(4):
            expand_x(s, 0)
            expand_x(s, 1)
        for s in range(4):
            expand_x(s, 2)
        for s in range(4, 8):
            expand_x(s, 0)
            expand_x(s, 1)
        for s in range(4, 8):
            expand_x(s, 2)

        stage = spool.tile([128, 3, 2, BPC], f32)
        stage2 = spool.tile([128, 3, 2, BPC], f32)

        def rhs_for(s, ct, nmm):
            g, o = divmod(ct, 2)
            return xts[s, g][:, 2 * o : 2 * o + 2, :nmm]

        # Phases: (br0..2, samples 0-3) then (br0..2, samples 4-7).
        for sh in range(2):
            for br in range(3):
                n = NS[br]
                nw = SEQ - n  # windows the reference maxes over
                nmm = nw + (nw & 1)  # keep the moving count even
                nt = NT2[br]
                ss = list(range(4 * sh, 4 * sh + 4))
                for si, s in enumerate(ss):
                    for dh in range(2):
                        ps = pspool.tile(
                            [128, 512], f32, tag="ps", name=f"ps_{br}_{s}_{dh}"
                        )
                        for ct in range(nt):
                            nc.tensor.matmul(
                                ps[:, :nmm],
                                lhsT=lhsT_for(dh, br, ct),
                                rhs=rhs_for(s, ct, nmm),
                                start=(ct == 0),
                                stop=(ct == nt - 1),
                                perf_mode=DR,
                            )
                        nc.vector.reduce_max(
                            stage[:, br, dh, s : s + 1],
                            ps[:, :nw],
                            axis=mybir.AxisListType.X,
                        )
        nc.vector.tensor_tensor(
            stage2[:],
            stage[:],
            bt[:, :, :, None].to_broadcast((128, 3, 2, BPC)),
            mybir.AluOpType.add,
        )
        nc.vector.tensor_scalar_max(stage2[:], stage2[:], 0.0)
        ring.dma_start(out_d[:], stage2[:])

    nc.compile()
    return nc


def _pack_inputs(input, W1, W2, W3, b1, b2, b3):
    f8 = ml_dtypes.float8_e4m3
    # Unreplicated transposed x: xb[b, p, blk, t] = x[b, t, 128*blk + p],
    # SEQ padded to 400 and rows >= 300 zero (the on-chip im2col expansion
    # reads the zero rows for K-tile 11's tail).
    xt_t = np.zeros((B, EMB, SEQP), np.float32)
    xt_t[:, :, :SEQ] = np.asarray(input, np.float32).transpose(0, 2, 1)
    xb = np.zeros((B, 128, 3, SEQP), np.float32)
    for blk in range(3):
        n = min(128, EMB - 128 * blk)
        xb[:, :n, blk] = xt_t[:, 128 * blk : 128 * blk + n]
    xt = xb.astype(f8)

    wq = np.zeros((128, 2, NCT, 2, 128), np.float32)
    for br, (n, W) in enumerate(zip(NS, (W1, W2, W3))):
        nt = NT2[br]
        Wt = np.zeros((nt * 256, DEPTH), np.float32)
        Wt[: n * EMB] = np.asarray(W, np.float32).T
        Wr = Wt.reshape(nt, 2, 128, 2, 128)  # [ct, i, p, dh, m]
        wq[:, :, CTB[br] : CTB[br] + nt] = Wr.transpose(2, 3, 0, 1, 4)
    wq = wq.astype(f8)

    bp = np.empty((128, 3, 2), np.float32)
    for br, b in enumerate((b1, b2, b3)):
        b = np.asarray(b, np.float32).reshape(DEPTH)
        for dh in range(2):
            bp[:, br, dh] = b[dh * 128 : (dh + 1) * 128]
    return xt, wq, bp


def kernel(input, W1, W2, W3, b1, b2, b3):
    global _built, LAST_RESULT
    from concourse.bass_utils import run_bass_kernel_spmd

    xt, wq, bp = _pack_inputs(input, W1, W2, W3, b1, b2, b3)

    if _built is None:
        _built = _build_bass()
    nc = _built

    in_maps = [
        {"xb": xt[c * BPC : (c + 1) * BPC], "wq": wq, "bp": bp}
        for c in range(NCORES)
    ]
    res = run_bass_kernel_spmd(
        nc, in_maps, core_ids=list(range(NCORES)), trace=TRACE
    )
    LAST_RESULT = res

    out = np.empty((B, 3 * DEPTH), np.float32)
    for c in range(NCORES):
        arr = res.results[c]["out_t"]  # [128, 3, 2, BPC]
        out[c * BPC : (c + 1) * BPC] = arr.transpose(3, 1, 2, 0).reshape(BPC, 768)
    return out


# revision 28
# speedup vs baseline: 1.6163x; 1.6163x over previous
"""TextCNN-style conv layer (kernel sizes 3/4/5, EMB=300 -> DEPTH=256, bias,
ReLU, max-pool over time) as a Bass/Tile kernel for 8 Trainium2 NeuronCores.

Strategy: data-parallel over batch (8 samples per core), weights replicated.

Conv as dense-K matmuls over the im2col matrix Xrep[k, i] = x[i + k//300,
k%300] in fp8 e4m3 with perf_mode=DoubleRow: each matmul contracts 256
virtual K-rows (two 128-row blocks presented as [128, 2, N] APs, 2 fp8 MACs
per PE cell per cycle), so a branch needs ceil(n*300/256) matmuls (4/5/6 for
n=3/4/5).  Measured MM pace is N/2.4GHz (165ns at N=390) with LDWEIGHTS
fully hidden, i.e. a 240-matmul / ~40us PE floor per core.  Branch
boundaries inside a super-tile are zero-padded in the *weights*.

End-to-end e4m3 quantization error on this distribution measures ~1.2e-2
relative L2, inside the 2e-2 gate.

Schedule (from trace analysis of v1):
- Phases (branch, sample-half); all s0-3 phases first so the early DMA only
  races compute for half the x bytes.  Within a phase, per-(sample,
  depth-half) K-contiguous matmul chains -> stop-matmuls arrive ~1us apart
  and the DVE reduce_max (561ns each) never bursts or stalls the PE.
- x is packed as 3 tiles per sample ([128, 4, SEQP] fp8 = two DoubleRow
  super-tiles each) to cut DMA/semaphore/teardown count while keeping the
  first-need granularity small; issue order == first-need order on the two
  fast HWDGE rings; br1/br2 weights + bias ride the gpsimd SWDGE ring.
- ~10 junk matmuls on a memset tile warm the PE HAM clock gate (1.2->2.4
  GHz) during the unavoidable initial DMA wait.

Epilogue: relu(max_i(y + b)) == max(0, max_i y + b): DVE reduce_max over the
window axis straight out of PSUM, broadcast bias add + clamp at 0, output
staged [d, branch, half, sample] per core and de-transposed on host.
"""

import numpy as np
import ml_dtypes

B, SEQ, EMB = 64, 394, 300
DEPTH = 256
NCORES = 8
BPC = B // NCORES  # samples per core
SEQP = 400  # free-dim padded (zeros) so shifted loads stay in bounds
NS = (3, 4, 5)
NT2 = (4, 5, 6)  # ceil(n*300/256) 256-row K-super-tiles per branch
CTB = (0, 4, 9)  # weight super-tile base per branch in the packed array
NCT = 15
KCT = 6  # distinct Xrep 256-row super-tiles per sample
XG = 3  # x tile groups per sample (2 super-tiles each)
NWARM = 8

TRACE = False
LAST_RESULT = None

_built = None


def _build_bass():
    import concourse.mybir as mybir
    import concourse.tile as tile
    from concourse import bacc
    from contextlib import ExitStack

    f32 = mybir.dt.float32
    f8 = mybir.dt.float8e4
    DR = mybir.MatmulPerfMode.DoubleRow

    nc = bacc.Bacc("TRN2", target_bir_lowering=False)
    xt_d = nc.dram_tensor("xt", (BPC, XG, 128, 4, SEQP), f8, kind="ExternalInput")
    wq_d = nc.dram_tensor("wq", (128, 2, NCT, 2, 128), f8, kind="ExternalInput")
    bp_d = nc.dram_tensor("bp", (128, 3, 2), f32, kind="ExternalInput")
    out_d = nc.dram_tensor("out_t", (128, 3, 2, BPC), f32, kind="ExternalOutput")

    with tile.TileContext(nc) as tc, ExitStack() as ctx:
        xpool = ctx.enter_context(tc.tile_pool(name="x", bufs=1))
        wpool = ctx.enter_context(tc.tile_pool(name="w", bufs=1))
        cpool = ctx.enter_context(tc.tile_pool(name="consts", bufs=1))
        spool = ctx.enter_context(tc.tile_pool(name="stage", bufs=1))
        pspool = ctx.enter_context(tc.tile_pool(name="ps", bufs=8, space="PSUM"))

        # PE warmup: junk DoubleRow matmuls on a zeroed tile, no DMA deps.
        # The PSUM bank is never read; the first real matmul on that bank
        # uses start=True, which overwrites.
        wz = cpool.tile([128, 2, SEQP], f8)
        nc.gpsimd.memset(wz[:], 0)
        psw = pspool.tile([128, 512], f32, tag="ps", name="ps_warm")
        for i in range(NWARM):
            nc.tensor.matmul(
                psw[:, :390],
                lhsT=wz[:, :, :128],
                rhs=wz[:, :, :390],
                start=True,
                stop=True,
                perf_mode=DR,
            )

        # Single DMA ring: the Tile runtime allocates semaphores per
        # (ring x DMA queue); using one ring cuts the fixed start/end
        # semaphore-clear chains (~57 -> ~25 waits per engine).
        ring = nc.sync

        wts = {}

        def load_w(dh, br):
            nt = NT2[br]
            wt = wpool.tile([128, nt, 2, 128], f8, tag=f"w{dh}{br}")
            ring.dma_start(wt[:], wq_d[:, dh, CTB[br] : CTB[br] + nt])
            wts[dh, br] = wt

        def lhsT_for(dh, br, ct):
            return wts[dh, br][:, ct]

        xts = {}

        def load_x(s, g):
            xtile = xpool.tile([128, 4, SEQP], f8, tag=f"x{s}_{g}")
            ring.dma_start(xtile[:], xt_d[s, g])
            xts[s, g] = xtile

        # DMA issue order == first-need order.
        load_w(0, 0)
        for s in range(4):
            load_x(s, 0)
            if s == 0:
                load_w(1, 0)
            load_x(s, 1)
        load_w(0, 1)
        load_w(1, 1)
        for s in range(4):
            load_x(s, 2)
        load_w(0, 2)
        load_w(1, 2)
        for s in range(4, 8):
            load_x(s, 0)
            load_x(s, 1)
        for s in range(4, 8):
            load_x(s, 2)

        bt = cpool.tile([128, 3, 2], f32)
        ring.dma_start(bt[:], bp_d[:])

        stage = spool.tile([128, 3, 2, BPC], f32)
        stage2 = spool.tile([128, 3, 2, BPC], f32)

        def rhs_for(s, ct, nmm):
            g, o = divmod(ct, 2)
            return xts[s, g][:, 2 * o : 2 * o + 2, :nmm]

        # Phases: (br0..2, samples 0-3) then (br0..2, samples 4-7).
        for sh in range(2):
            for br in range(3):
                n = NS[br]
                nw = SEQ - n  # windows the reference maxes over
                nmm = nw + (nw & 1)  # keep the moving count even
                nt = NT2[br]
                ss = list(range(4 * sh, 4 * sh + 4))
                for si, s in enumerate(ss):
                    for dh in range(2):
                        ps = pspool.tile(
                            [128, 512], f32, tag="ps", name=f"ps_{br}_{s}_{dh}"
                        )
                        for ct in range(nt):
                            nc.tensor.matmul(
                                ps[:, :nmm],
                                lhsT=lhsT_for(dh, br, ct),
                                rhs=rhs_for(s, ct, nmm),
                                start=(ct == 0),
                                stop=(ct == nt - 1),
                                perf_mode=DR,
                            )
                        nc.vector.reduce_max(
                            stage[:, br, dh, s : s + 1],
                            ps[:, :nw],
                            axis=mybir.AxisListType.X,
                        )
        nc.vector.tensor_tensor(
            stage2[:],
            stage[:],
            bt[:, :, :, None].to_broadcast((128, 3, 2, BPC)),
            mybir.AluOpType.add,
        )
        nc.vector.tensor_scalar_max(stage2[:], stage2[:], 0.0)
        ring.dma_start(out_d[:], stage2[:])

    nc.compile()
    return nc


def _pack_inputs(input, W1, W2, W3, b1, b2, b3):
    f8 = ml_dtypes.float8_e4m3
    # Host-materialized im2col: Xrep[b, k, t] = x[b, t + k//300, k%300], laid
    # out as 3 groups of [128 partitions, 4 blocks, SEQP], virtual row
    # k = 512*g + 128*j + p.  SEQ padded to 400 with zeros; rows >= 1500
    # stay zero.
    xt_t = np.zeros((B, EMB, SEQP), np.float32)
    xt_t[:, :, :SEQ] = np.asarray(input, np.float32).transpose(0, 2, 1)
    xrep = np.zeros((B, KCT * 256, SEQP), np.float32)
    for j in range(5):
        xrep[:, j * EMB : (j + 1) * EMB, : SEQP - j] = xt_t[:, :, j:]
    xt = xrep.reshape(B, XG, 4, 128, SEQP).transpose(0, 1, 3, 2, 4)
    xt = np.ascontiguousarray(xt).astype(f8)

    wq = np.zeros((128, 2, NCT, 2, 128), np.float32)
    for br, (n, W) in enumerate(zip(NS, (W1, W2, W3))):
        nt = NT2[br]
        Wt = np.zeros((nt * 256, DEPTH), np.float32)
        Wt[: n * EMB] = np.asarray(W, np.float32).T
        Wr = Wt.reshape(nt, 2, 128, 2, 128)  # [ct, i, p, dh, m]
        wq[:, :, CTB[br] : CTB[br] + nt] = Wr.transpose(2, 3, 0, 1, 4)
    wq = wq.astype(f8)

    bp = np.empty((128, 3, 2), np.float32)
    for br, b in enumerate((b1, b2, b3)):
        b = np.asarray(b, np.float32).reshape(DEPTH)
        for dh in range(2):
            bp[:, br, dh] = b[dh * 128 : (dh + 1) * 128]
    return xt, wq, bp


def kernel(input, W1, W2, W3, b1, b2, b3):
    global _built, LAST_RESULT
    from concourse.bass_utils import run_bass_kernel_spmd

    xt, wq, bp = _pack_inputs(input, W1, W2, W3, b1, b2, b3)

    if _built is None:
        _built = _build_bass()
    nc = _built

    in_maps = [
        {"xt": xt[c * BPC : (c + 1) * BPC], "wq": wq, "bp": bp}
        for c in range(NCORES)
    ]
    res = run_bass_kernel_spmd(
        nc, in_maps, core_ids=list(range(NCORES)), trace=TRACE
    )
    LAST_RESULT = res

    out = np.empty((B, 3 * DEPTH), np.float32)
    for c in range(NCORES):
        arr = res.results[c]["out_t"]  # [128, 3, 2, BPC]
        out[c * BPC : (c + 1) * BPC] = arr.transpose(3, 1, 2, 0).reshape(BPC, 768)
    return out
